# revision 11
# baseline (speedup 1.0000x reference)
"""Trainium2 Bass kernel for per-joint vertex-MLP regression (SMPL-style).

Problem: for each of J=24 joints, gather n_j support vertices from
vertices[B, V, 3], mean-center, run a 2-layer MLP on the flattened
[B, 3n_j] features, add the mean back -> output [B, J, 3].

Strategy (per the data-parallel sharding hint):
  - shard batch B=2048 across 8 NeuronCores (256 each), replicate weights
  - host-side: fold the mean-centering into W1 (W1' = W1 (I - P M)) and
    append the 3 mean rows (M) as extra output rows of layer 1, so the
    device only does: gather -> matmul -> relu -> matmul (+mean via I3).
  - device phase 1: PE-transpose vertices [256, 3V] -> v_packed [V, 3*256]
    (feature-major) in DRAM scratch.
  - device phase 2: per joint, one SWDGE dma_gather pulls the n_j support
    rows ([3,256] blocks) into SBUF; bf16 matmuls accumulate in fp32 PSUM.

All gather offsets / shapes are compile-time constants (joint_idx is a
constant input), derived from the inputs at kernel() call time.
"""

import math
import sys
from functools import lru_cache

import numpy as np

sys.path.insert(0, "/opt/trn_rl_repo")

import ml_dtypes  # noqa: E402

BF16 = ml_dtypes.bfloat16

P = 128  # partitions


def _cdiv(a, b):
    return -(-a // b)


# ---------------------------------------------------------------------------
# Host-side packing
# ---------------------------------------------------------------------------

def _pack_weights(W1s, b1s, W2s, b2s, joint_idx, V):
    """Fold mean-centering into W1, reorder contraction dim to gather-slot
    order, and pack everything into flat blobs with per-(j,m) offsets."""
    J = len(joint_idx)
    meta = {"J": J, "V": V, "joints": []}

    w1_cols, w2_cols, b1_cols, idx_cols, mp_cols = [], [], [], [], []
    w1_off = 0
    w2_grp = 0  # running count of (j, m) groups of 3 cols in w2 blob
    b1_col = 0
    idx_off = 0  # in units of 16-wide column groups (8 per 128 idxs)
    mp_off = 0  # column offset into mp blob

    for j in range(J):
        idx = np.asarray(joint_idx[j], np.int64)
        n = int(idx.shape[0])
        d = 3 * n
        cn = _cdiv(n, P)
        Mt = _cdiv(d, P)

        W1 = np.asarray(W1s[j], np.float32).astype(np.float64)  # [d, d]
        b1 = np.asarray(b1s[j], np.float32)
        W2 = np.asarray(W2s[j], np.float32).astype(np.float64)  # [3, d]
        b2 = np.asarray(b2s[j], np.float32)

        # W1'[i, 3k+c] = W1[i, 3k+c] - (sum_k' W1[i, 3k'+c]) / n
        W1r = W1.reshape(d, n, 3)
        A = W1r.sum(axis=1)  # [d, 3]
        W1p = (W1r - A[:, None, :] / n).reshape(d, d)

        # contraction dim -> (slot, coord), pad slots to 128*cn with zeros
        Wq = np.zeros((d, P * cn, 3))
        Wq[:, :n, :] = W1p.reshape(d, n, 3)

        jm = []
        for m in range(Mt):
            mrows = min(P, d - P * m)
            mcols = mrows
            rows = list(range(P * m, P * m + mrows))
            sel = Wq[rows]  # [mcols, 128*cn, 3]
            arr = (
                sel.reshape(mcols, cn, P, 3)
                .transpose(2, 1, 3, 0)  # [128 p, cn g, 3 c, mcols]
                .reshape(P, cn * 3 * mcols)
            )
            w1_cols.append(arr)

            w2c = np.zeros((P, 3))
            w2c[:mrows, :] = W2[:, P * m : P * m + mrows].T
            w2_cols.append(w2c)

            b1c = np.zeros((P, 1), np.float32)
            b1c[:mrows, 0] = b1[P * m : P * m + mrows]
            b1_cols.append(b1c)

            jm.append(
                dict(
                    w1_off=w1_off,
                    w1_w=cn * 3 * mcols,
                    mrows=mrows,
                    mcols=mcols,
                    w2_grp=w2_grp,
                    b1_col=b1_col,
                )
            )
            w1_off += cn * 3 * mcols
            w2_grp += 1
            b1_col += 1

        # mean blob: per (g, c) a [128, 3] tile, Mp[p, c'] = (1/n)[c'==c][slot<n]
        mp = np.zeros((P, cn, 3, 3))
        for g in range(cn):
            valid = max(0, min(P, n - P * g))
            for c in range(3):
                mp[:valid, g, c, c] = 1.0 / n
        mp_cols.append(mp.reshape(P, cn * 9))

        # idx blob: pad to 128*cn with 0; wrap into 16 partitions, 8 copies
        idx_pad = np.zeros(P * cn, np.int16)
        idx_pad[:n] = idx.astype(np.int16)
        cols16 = idx_pad.reshape(8 * cn, 16).T  # [16, 8*cn]
        idx_cols.append(np.tile(cols16, (8, 1)))  # [128, 8*cn]

        meta["joints"].append(
            dict(n=n, d=d, cn=cn, Mt=Mt, idx_off=idx_off, mp_off=mp_off, jm=jm, b2=b2)
        )
        idx_off += 8 * cn
        mp_off += cn * 9

    w1blob = np.concatenate(w1_cols, axis=1).astype(BF16)
    w2blob = np.concatenate(w2_cols, axis=1).astype(BF16)
    b1blob = np.concatenate(b1_cols, axis=1).astype(np.float32)
    b2blob = np.stack([jj["b2"] for jj in meta["joints"]], axis=1).astype(
        np.float32
    )  # [3, J]
    idxblob = np.concatenate(idx_cols, axis=1).astype(np.int16)
    mpblob = np.concatenate(mp_cols, axis=1).astype(BF16)

    meta["tot1"] = w1blob.shape[1]
    meta["tot2"] = w2blob.shape[1]
    meta["totb1"] = b1blob.shape[1]
    meta["totidx"] = idxblob.shape[1]
    meta["totmp"] = mpblob.shape[1]
    blobs = dict(
        w1blob=w1blob,
        w2blob=w2blob,
        b1blob=b1blob,
        b2blob=b2blob,
        idxblob=idxblob,
        mpblob=mpblob,
    )
    return meta, blobs


# ---------------------------------------------------------------------------
# Device program
# ---------------------------------------------------------------------------

def _build_program(meta, b_shard):
    import concourse.mybir as mybir
    import concourse.tile as tile
    from concourse import bacc
    from concourse.masks import make_identity

    dt = mybir.dt
    V = meta["V"]
    J = meta["J"]
    F = 3 * V
    NBT = _cdiv(b_shard, P)  # batch tiles (2 for 256)
    assert b_shard % P == 0
    NSUB_TOT = _cdiv(F, P)
    Vpad = _cdiv(NSUB_TOT * P, 3)

    nc = bacc.Bacc("TRN2", target_bir_lowering=False, debug=False)

    verts = nc.dram_tensor("verts", [b_shard, F], dt.float32, kind="ExternalInput").ap()
    w1blob = nc.dram_tensor(
        "w1blob", [P, meta["tot1"]], dt.bfloat16, kind="ExternalInput"
    ).ap()
    w2blob = nc.dram_tensor(
        "w2blob", [P, meta["tot2"]], dt.bfloat16, kind="ExternalInput"
    ).ap()
    b1blob = nc.dram_tensor(
        "b1blob", [P, meta["totb1"]], dt.float32, kind="ExternalInput"
    ).ap()
    b2blob = nc.dram_tensor("b2blob", [3, J], dt.float32, kind="ExternalInput").ap()
    idxblob = nc.dram_tensor(
        "idxblob", [P, meta["totidx"]], dt.int16, kind="ExternalInput"
    ).ap()
    mpblob = nc.dram_tensor(
        "mpblob", [P, meta["totmp"]], dt.bfloat16, kind="ExternalInput"
    ).ap()
    out = nc.dram_tensor("out", [J, 3, b_shard], dt.float32, kind="ExternalOutput").ap()

    CH = 2560  # feature columns per load DMA (multiple of 128)

    with tile.TileContext(nc) as tc:
        with (
            tc.tile_pool(name="consts", bufs=1) as consts,
            tc.tile_pool(name="dram", bufs=1, space="DRAM") as dram,
            tc.tile_pool(name="ld", bufs=4) as ldpool,
            tc.tile_pool(name="vt", bufs=4) as vtpool,
            tc.tile_pool(name="ps1", bufs=2, space="PSUM") as ps1pool,
            tc.tile_pool(name="w1", bufs=8) as w1pool,
            tc.tile_pool(name="vg", bufs=3) as vgpool,
            tc.tile_pool(name="a1", bufs=2) as a1pool,
            tc.tile_pool(name="pa", bufs=4, space="PSUM") as papool,
            tc.tile_pool(name="pe2", bufs=2, space="PSUM") as pe2pool,
        ):
            # ---- constants ----
            w2_t = consts.tile([P, meta["tot2"]], dt.bfloat16)
            nc.sync.dma_start(w2_t, w2blob)
            b1_t = consts.tile([P, meta["totb1"]], dt.float32)
            nc.sync.dma_start(b1_t, b1blob)
            b2_t = consts.tile([3, J], dt.float32)
            nc.sync.dma_start(b2_t, b2blob)
            idx_t = consts.tile([P, meta["totidx"]], dt.int16)
            nc.sync.dma_start(idx_t, idxblob)
            mp_t = consts.tile([P, meta["totmp"]], dt.bfloat16)
            nc.sync.dma_start(mp_t, mpblob)
            est_t = consts.tile([3, J * b_shard], dt.float32)

            ident = consts.tile([P, P], dt.float32)
            make_identity(nc, ident)

            # DRAM scratch: v_packed[v, c, b] = vertices[b, v, c]
            vp = dram.tile([Vpad, 3 * b_shard], dt.bfloat16)
            vp_rows = vp.rearrange("v (c b) -> (v c) b", c=3)  # [3*Vpad, b_shard]

            # ---- phase 1: transpose to feature-major ----
            ch0 = 0
            while ch0 < F:
                ch = min(CH, F - ch0)
                lds = []
                for bt in range(NBT):
                    ld = ldpool.tile([P, CH], dt.float32, tag="ld")
                    nc.sync.dma_start(
                        ld[:, :ch], verts[bt * P : (bt + 1) * P, ch0 : ch0 + ch]
                    )
                    lds.append(ld)
                for sub in range(_cdiv(ch, P)):
                    cw = min(P, ch - P * sub)
                    vt = vtpool.tile([P, b_shard], dt.bfloat16)
                    for bt in range(NBT):
                        ps = ps1pool.tile([P, P], dt.float32)
                        nc.tensor.transpose(
                            ps[:cw, :], lds[bt][:, P * sub : P * sub + cw], ident
                        )
                        nc.vector.tensor_copy(
                            vt[:cw, bt * P : (bt + 1) * P], ps[:cw, :]
                        )
                    r0 = ch0 + P * sub
                    nc.sync.dma_start(vp_rows[r0 : r0 + cw, :], vt[:cw, :])
                ch0 += ch

            pad_rows = 3 * Vpad - F
            if pad_rows > 0:
                zt = vtpool.tile([P, b_shard], dt.bfloat16)
                nc.vector.memset(zt[:pad_rows, :], 0.0)
                nc.sync.dma_start(vp_rows[F : F + pad_rows, :], zt[:pad_rows, :])

            # ---- phase 2: per-joint gather + MLP ----
            vp_g = vp  # [Vpad, 768] row-gather view
            for j in range(J):
                jj = meta["joints"][j]
                n, d, cn, Mt = jj["n"], jj["d"], jj["cn"], jj["Mt"]
                io = jj["idx_off"]

                vg = vgpool.tile([P, cn, 3 * b_shard], dt.bfloat16, tag="vg")
                nc.gpsimd.dma_gather(
                    vg[:, :, :],
                    vp_g[:, :],
                    idx_t[:, io : io + 8 * cn],
                    num_idxs=P * cn,
                    num_idxs_reg=P * cn,
                    elem_size=3 * b_shard,
                )

                a1 = a1pool.tile([P, Mt, b_shard], dt.bfloat16, tag="a1")

                for m in range(Mt):
                    g1 = jj["jm"][m]
                    mrows, mcols = g1["mrows"], g1["mcols"]
                    w1t = w1pool.tile([P, g1["w1_w"]], dt.bfloat16, tag="w1")
                    nc.sync.dma_start(
                        w1t, w1blob[:, g1["w1_off"] : g1["w1_off"] + g1["w1_w"]]
                    )
                    pa = papool.tile([P, b_shard], dt.float32)
                    kk = 0
                    for g in range(cn):
                        for c in range(3):
                            nc.tensor.matmul(
                                pa[:mcols, :],
                                w1t[:, (3 * g + c) * mcols : (3 * g + c + 1) * mcols],
                                vg[:, g, b_shard * c : b_shard * (c + 1)],
                                start=(kk == 0),
                                stop=(kk == 3 * cn - 1),
                            )
                            kk += 1
                    nc.scalar.activation(
                        a1[:mrows, m, :],
                        pa[:mrows, :],
                        mybir.ActivationFunctionType.Relu,
                        bias=b1_t[:mrows, g1["b1_col"] : g1["b1_col"] + 1],
                    )

                pe2 = pe2pool.tile([3, b_shard], dt.float32)
                for m in range(Mt):
                    g1 = jj["jm"][m]
                    mrows = g1["mrows"]
                    w2c = 3 * g1["w2_grp"]
                    nc.tensor.matmul(
                        pe2[:, :],
                        w2_t[:mrows, w2c : w2c + 3],
                        a1[:mrows, m, :],
                        start=(m == 0),
                        stop=False,
                    )
                mo = jj["mp_off"]
                for g in range(cn):
                    for c in range(3):
                        nc.tensor.matmul(
                            pe2[:, :],
                            mp_t[:, mo + 3 * (3 * g + c) : mo + 3 * (3 * g + c) + 3],
                            vg[:, g, b_shard * c : b_shard * (c + 1)],
                            start=False,
                            stop=(g == cn - 1 and c == 2),
                        )
                nc.scalar.activation(
                    est_t[:, b_shard * j : b_shard * (j + 1)],
                    pe2[:, :],
                    mybir.ActivationFunctionType.Identity,
                    bias=b2_t[:, j : j + 1],
                )

            est_v = est_t.rearrange("c (j b) -> c j b", j=J)
            nc.sync.dma_start(out.rearrange("j c b -> c j b"), est_v)

    nc.compile()
    return nc


# ---------------------------------------------------------------------------
# Entry point
# ---------------------------------------------------------------------------

_CACHE = {}


def _get_program(meta_key, meta, b_shard):
    if meta_key not in _CACHE:
        _CACHE[meta_key] = _build_program(meta, b_shard)
    return _CACHE[meta_key]


def kernel(vertices, W1s, b1s, W2s, b2s, joint_idx, _trace=False):
    from concourse.bass_utils import run_bass_kernel_spmd

    vertices = np.asarray(vertices, np.float32)
    B, V, _ = vertices.shape
    n_cores = 8
    assert B % n_cores == 0
    b_shard = B // n_cores

    meta, blobs = _pack_weights(W1s, b1s, W2s, b2s, joint_idx, V)
    meta_key = (B, V, tuple(jj["n"] for jj in meta["joints"]))
    nc = _get_program(meta_key, meta, b_shard)

    in_maps = []
    for core in range(n_cores):
        stripe = np.ascontiguousarray(
            vertices[core * b_shard : (core + 1) * b_shard].reshape(b_shard, 3 * V)
        )
        in_maps.append(dict(verts=stripe, **blobs))

    res = run_bass_kernel_spmd(
        nc, in_maps, core_ids=list(range(n_cores)), trace=_trace
    )
    J = meta["J"]
    full = np.empty((B, J, 3), np.float32)
    for core in range(n_cores):
        stage = res.results[core]["out"]  # [J, 3, b_shard]
        full[core * b_shard : (core + 1) * b_shard] = stage.transpose(2, 0, 1)
    if _trace:
        kernel._last_result = res
    return full


# revision 19
# speedup vs baseline: 1.0415x; 1.0415x over previous
"""Trainium2 Bass kernel for per-joint vertex-MLP regression (SMPL-style).

Problem: for each of J=24 joints, gather n_j support vertices from
vertices[B, V, 3], mean-center, run a 2-layer MLP on the flattened
[B, 3n_j] features, add the mean back -> output [B, J, 3].

Strategy (per the data-parallel sharding hint):
  - shard batch B=2048 across 8 NeuronCores (256 each), replicate weights
  - host-side: fold the mean-centering into W1 (W1' = W1 (I - P M)) and
    append the 3 mean rows (M) as extra output rows of layer 1, so the
    device only does: gather -> matmul -> relu -> matmul (+mean via I3).
  - device phase 1: PE-transpose vertices [256, 3V] -> v_packed [V, 3*256]
    (feature-major) in DRAM scratch.
  - device phase 2: per joint, one SWDGE dma_gather pulls the n_j support
    rows ([3,256] blocks) into SBUF; bf16 matmuls accumulate in fp32 PSUM.

All gather offsets / shapes are compile-time constants (joint_idx is a
constant input), derived from the inputs at kernel() call time.
"""

import math
import sys
from functools import lru_cache

import numpy as np

sys.path.insert(0, "/opt/trn_rl_repo")

import ml_dtypes  # noqa: E402

BF16 = ml_dtypes.bfloat16

P = 128  # partitions


def _cdiv(a, b):
    return -(-a // b)


# ---------------------------------------------------------------------------
# Host-side packing
# ---------------------------------------------------------------------------

def _pack_weights(W1s, b1s, W2s, b2s, joint_idx, V):
    """Fold mean-centering into W1, reorder contraction dim to gather-slot
    order, and pack everything into flat blobs with per-(j,m) offsets."""
    J = len(joint_idx)
    meta = {"J": J, "V": V, "joints": []}

    w1_cols, w2_cols, b1_cols, idx_cols, mp_cols = [], [], [], [], []
    w1_off = 0
    w2_grp = 0  # running count of (j, m) groups of 3 cols in w2 blob
    b1_col = 0
    idx_off = 0  # in units of 16-wide column groups (8 per 128 idxs)
    mp_off = 0  # column offset into mp blob

    for j in range(J):
        idx = np.asarray(joint_idx[j], np.int64)
        n = int(idx.shape[0])
        d = 3 * n
        cn = _cdiv(n, P)
        Mt = _cdiv(d, P)

        W1 = np.asarray(W1s[j], np.float32).astype(np.float64)  # [d, d]
        b1 = np.asarray(b1s[j], np.float32)
        W2 = np.asarray(W2s[j], np.float32).astype(np.float64)  # [3, d]
        b2 = np.asarray(b2s[j], np.float32)

        # W1'[i, 3k+c] = W1[i, 3k+c] - (sum_k' W1[i, 3k'+c]) / n
        W1r = W1.reshape(d, n, 3)
        A = W1r.sum(axis=1)  # [d, 3]
        W1p = (W1r - A[:, None, :] / n).reshape(d, d)

        # contraction dim -> (slot, coord), pad slots to 128*cn with zeros
        Wq = np.zeros((d, P * cn, 3))
        Wq[:, :n, :] = W1p.reshape(d, n, 3)

        jm = []
        for m in range(Mt):
            mrows = min(P, d - P * m)
            mcols = mrows
            rows = list(range(P * m, P * m + mrows))
            sel = Wq[rows]  # [mcols, 128*cn, 3]
            arr = (
                sel.reshape(mcols, cn, P, 3)
                .transpose(2, 1, 3, 0)  # [128 p, cn g, 3 c, mcols]
                .reshape(P, cn * 3 * mcols)
            )
            w1_cols.append(arr)

            w2c = np.zeros((P, 3))
            w2c[:mrows, :] = W2[:, P * m : P * m + mrows].T
            w2_cols.append(w2c)

            b1c = np.zeros((P, 1), np.float32)
            b1c[:mrows, 0] = b1[P * m : P * m + mrows]
            b1_cols.append(b1c)

            jm.append(
                dict(
                    w1_off=w1_off,
                    w1_w=cn * 3 * mcols,
                    mrows=mrows,
                    mcols=mcols,
                    w2_grp=w2_grp,
                    b1_col=b1_col,
                )
            )
            w1_off += cn * 3 * mcols
            w2_grp += 1
            b1_col += 1

        # mean blob: per (g, c) a [128, 3] tile, Mp[p, c'] = (1/n)[c'==c][slot<n]
        mp = np.zeros((P, cn, 3, 3))
        for g in range(cn):
            valid = max(0, min(P, n - P * g))
            for c in range(3):
                mp[:valid, g, c, c] = 1.0 / n
        mp_cols.append(mp.reshape(P, cn * 9))

        # idx blob: pad to 128*cn with 0; wrap into 16 partitions, 8 copies
        idx_pad = np.zeros(P * cn, np.int16)
        idx_pad[:n] = idx.astype(np.int16)
        cols16 = idx_pad.reshape(8 * cn, 16).T  # [16, 8*cn]
        idx_cols.append(np.tile(cols16, (8, 1)))  # [128, 8*cn]

        meta["joints"].append(
            dict(n=n, d=d, cn=cn, Mt=Mt, idx_off=idx_off, mp_off=mp_off, jm=jm, b2=b2)
        )
        idx_off += 8 * cn
        mp_off += cn * 9

    w1blob = np.concatenate(w1_cols, axis=1).astype(BF16)
    w2blob = np.concatenate(w2_cols, axis=1).astype(BF16)
    b1blob = np.concatenate(b1_cols, axis=1).astype(np.float32)
    b2blob = np.stack([jj["b2"] for jj in meta["joints"]], axis=1).astype(
        np.float32
    )  # [3, J]
    idxblob = np.concatenate(idx_cols, axis=1).astype(np.int16)
    mpblob = np.concatenate(mp_cols, axis=1).astype(BF16)

    meta["tot1"] = w1blob.shape[1]
    meta["tot2"] = w2blob.shape[1]
    meta["totb1"] = b1blob.shape[1]
    meta["totidx"] = idxblob.shape[1]
    meta["totmp"] = mpblob.shape[1]
    blobs = dict(
        w1blob=w1blob,
        w2blob=w2blob,
        b1blob=b1blob,
        b2blob=b2blob,
        idxblob=idxblob,
        mpblob=mpblob,
    )
    return meta, blobs


def _make_etrans(b_shard):
    """E[:, bt*b_shard : (bt+1)*b_shard] is the identity that routes batch
    tile bt's 128 rows to columns [bt*128, bt*128+128) of the transpose."""
    nbt = _cdiv(b_shard, P)
    E = np.zeros((P, nbt * b_shard), np.float32)
    for bt in range(nbt):
        for p in range(P):
            E[p, bt * b_shard + bt * P + p] = 1.0
    return E


# ---------------------------------------------------------------------------
# Device program
# ---------------------------------------------------------------------------

def _build_program(meta, b_shard):
    import concourse.mybir as mybir
    import concourse.tile as tile
    from concourse import bacc

    dt = mybir.dt
    V = meta["V"]
    J = meta["J"]
    F = 3 * V
    NBT = _cdiv(b_shard, P)  # batch tiles (2 for 256)
    assert b_shard % P == 0
    NSUB_TOT = _cdiv(F, P)
    Vpad = _cdiv(NSUB_TOT * P, 3)

    nc = bacc.Bacc("TRN2", target_bir_lowering=False, debug=False)

    # float32r: same bits as fp32 but the PE streams it at full rate when the
    # moving free dim is >= 256 (vs 4 cycles/row for plain fp32).
    verts = nc.dram_tensor(
        "verts", [b_shard, F], dt.float32r, kind="ExternalInput"
    ).ap()
    etrans = nc.dram_tensor(
        "etrans", [P, NBT * b_shard], dt.float32r, kind="ExternalInput"
    ).ap()
    w1blob = nc.dram_tensor(
        "w1blob", [P, meta["tot1"]], dt.bfloat16, kind="ExternalInput"
    ).ap()
    w2blob = nc.dram_tensor(
        "w2blob", [P, meta["tot2"]], dt.bfloat16, kind="ExternalInput"
    ).ap()
    b1blob = nc.dram_tensor(
        "b1blob", [P, meta["totb1"]], dt.float32, kind="ExternalInput"
    ).ap()
    b2blob = nc.dram_tensor("b2blob", [3, J], dt.float32, kind="ExternalInput").ap()
    idxblob = nc.dram_tensor(
        "idxblob", [P, meta["totidx"]], dt.int16, kind="ExternalInput"
    ).ap()
    mpblob = nc.dram_tensor(
        "mpblob", [P, meta["totmp"]], dt.bfloat16, kind="ExternalInput"
    ).ap()
    out = nc.dram_tensor("out", [J, 3, b_shard], dt.float32, kind="ExternalOutput").ap()

    CH = 2560  # feature columns per load DMA (multiple of 128)

    with tile.TileContext(nc) as tc:
        with (
            tc.tile_pool(name="consts", bufs=1) as consts,
            tc.tile_pool(name="dram", bufs=1, space="DRAM") as dram,
            tc.tile_pool(name="ld", bufs=4) as ldpool,
            tc.tile_pool(name="vt", bufs=6) as vtpool,
            tc.tile_pool(name="w1", bufs=12) as w1pool,
            tc.tile_pool(name="vg", bufs=3) as vgpool,
            tc.tile_pool(name="a1", bufs=2) as a1pool,
            tc.tile_pool(name="pa", bufs=6, space="PSUM") as papool,
            tc.tile_pool(name="pe2", bufs=2, space="PSUM") as pe2pool,
        ):
            # ---- constants ----
            w2_t = consts.tile([P, meta["tot2"]], dt.bfloat16)
            nc.sync.dma_start(w2_t, w2blob)
            b1_t = consts.tile([P, meta["totb1"]], dt.float32)
            nc.sync.dma_start(b1_t, b1blob)
            b2_t = consts.tile([3, J], dt.float32)
            nc.sync.dma_start(b2_t, b2blob)
            idx_t = consts.tile([P, meta["totidx"]], dt.int16)
            nc.sync.dma_start(idx_t, idxblob)
            mp_t = consts.tile([P, meta["totmp"]], dt.bfloat16)
            nc.sync.dma_start(mp_t, mpblob)
            est_t = consts.tile([3, J * b_shard], dt.float32)

            # DRAM scratch: v_packed[v, c, b] = vertices[b, v, c]
            vp = dram.tile([Vpad, 3 * b_shard], dt.bfloat16)
            vp_rows = vp.rearrange("v (c b) -> (v c) b", c=3)  # [3*Vpad, b_shard]
            Fpad = NSUB_TOT * P

            # ---- phase 1: transpose to feature-major ----
            # Transpose [128b, 128f] tiles on the PE as fp32r matmuls against
            # shifted identities (full rate at N=256, pipelines back-to-back);
            # both batch tiles accumulate into one [128f, 256b] PSUM tile.
            # Loads go on the SP HWDGE ring, stores on the ACT ring so the
            # two DMA streams don't serialize against each other.
            et_t = consts.tile([P, NBT * b_shard], dt.float32r)
            nc.sync.dma_start(et_t, etrans)

            ch0 = 0
            while ch0 < F:
                ch = min(CH, F - ch0)
                lds = []
                for bt in range(NBT):
                    ld = ldpool.tile([P, CH], dt.float32r, tag="ld")
                    nc.sync.dma_start(
                        ld[:, :ch], verts[bt * P : (bt + 1) * P, ch0 : ch0 + ch]
                    )
                    lds.append(ld)
                for sub in range(_cdiv(ch, P)):
                    cw = min(P, ch - P * sub)
                    ps = papool.tile([P, b_shard], dt.float32, tag="pa")
                    for bt in range(NBT):
                        nc.tensor.matmul(
                            ps[:cw, :],
                            lds[bt][:, P * sub : P * sub + cw],
                            et_t[:, bt * b_shard : (bt + 1) * b_shard],
                            start=(bt == 0),
                            stop=(bt == NBT - 1),
                        )
                    vt = vtpool.tile([P, b_shard], dt.bfloat16)
                    nc.vector.tensor_copy(vt[:cw, :], ps[:cw, :])
                    f0 = ch0 + P * sub
                    nc.scalar.dma_start(vp_rows[f0 : f0 + cw, :], vt[:cw, :])
                ch0 += ch

            pad_rows = 3 * Vpad - F
            if pad_rows > 0:
                zt = vtpool.tile([P, b_shard], dt.bfloat16, tag="zt")
                nc.vector.memset(zt[:, :], 0.0)
                nc.scalar.dma_start(vp_rows[F : 3 * Vpad, :], zt[:pad_rows, :])

            # ---- phase 2: per-joint gather + MLP ----
            vp_g = vp  # [Vpad, 768] row-gather view
            for j in range(J):
                jj = meta["joints"][j]
                n, d, cn, Mt = jj["n"], jj["d"], jj["cn"], jj["Mt"]
                io = jj["idx_off"]

                vg = vgpool.tile([P, cn, 3 * b_shard], dt.bfloat16, tag="vg")
                nc.gpsimd.dma_gather(
                    vg[:, :, :],
                    vp_g[:, :],
                    idx_t[:, io : io + 8 * cn],
                    num_idxs=P * cn,
                    num_idxs_reg=P * cn,
                    elem_size=3 * b_shard,
                )

                a1 = a1pool.tile([P, Mt, b_shard], dt.bfloat16, tag="a1")

                for m in range(Mt):
                    g1 = jj["jm"][m]
                    mrows, mcols = g1["mrows"], g1["mcols"]
                    w1t = w1pool.tile([P, g1["w1_w"]], dt.bfloat16, tag="w1")
                    nc.sync.dma_start(
                        w1t, w1blob[:, g1["w1_off"] : g1["w1_off"] + g1["w1_w"]]
                    )
                    pa = papool.tile([P, b_shard], dt.float32)
                    kk = 0
                    for g in range(cn):
                        for c in range(3):
                            nc.tensor.matmul(
                                pa[:mcols, :],
                                w1t[:, (3 * g + c) * mcols : (3 * g + c + 1) * mcols],
                                vg[:, g, b_shard * c : b_shard * (c + 1)],
                                start=(kk == 0),
                                stop=(kk == 3 * cn - 1),
                            )
                            kk += 1
                    nc.scalar.activation(
                        a1[:mrows, m, :],
                        pa[:mrows, :],
                        mybir.ActivationFunctionType.Relu,
                        bias=b1_t[:mrows, g1["b1_col"] : g1["b1_col"] + 1],
                    )

                pe2 = pe2pool.tile([3, b_shard], dt.float32)
                for m in range(Mt):
                    g1 = jj["jm"][m]
                    mrows = g1["mrows"]
                    w2c = 3 * g1["w2_grp"]
                    nc.tensor.matmul(
                        pe2[:, :],
                        w2_t[:mrows, w2c : w2c + 3],
                        a1[:mrows, m, :],
                        start=(m == 0),
                        stop=False,
                    )
                mo = jj["mp_off"]
                for g in range(cn):
                    for c in range(3):
                        nc.tensor.matmul(
                            pe2[:, :],
                            mp_t[:, mo + 3 * (3 * g + c) : mo + 3 * (3 * g + c) + 3],
                            vg[:, g, b_shard * c : b_shard * (c + 1)],
                            start=False,
                            stop=(g == cn - 1 and c == 2),
                        )
                nc.scalar.activation(
                    est_t[:, b_shard * j : b_shard * (j + 1)],
                    pe2[:, :],
                    mybir.ActivationFunctionType.Identity,
                    bias=b2_t[:, j : j + 1],
                )

            est_v = est_t.rearrange("c (j b) -> c j b", j=J)
            nc.sync.dma_start(out.rearrange("j c b -> c j b"), est_v)

    nc.compile()
    return nc


# ---------------------------------------------------------------------------
# Entry point
# ---------------------------------------------------------------------------

_CACHE = {}


def _get_program(meta_key, meta, b_shard):
    if meta_key not in _CACHE:
        _CACHE[meta_key] = _build_program(meta, b_shard)
    return _CACHE[meta_key]


def kernel(vertices, W1s, b1s, W2s, b2s, joint_idx, _trace=False):
    from concourse.bass_utils import run_bass_kernel_spmd

    vertices = np.asarray(vertices, np.float32)
    B, V, _ = vertices.shape
    n_cores = 8
    assert B % n_cores == 0
    b_shard = B // n_cores

    meta, blobs = _pack_weights(W1s, b1s, W2s, b2s, joint_idx, V)
    blobs["etrans"] = _make_etrans(b_shard)
    meta_key = (B, V, tuple(jj["n"] for jj in meta["joints"]))
    nc = _get_program(meta_key, meta, b_shard)

    in_maps = []
    for core in range(n_cores):
        stripe = np.ascontiguousarray(
            vertices[core * b_shard : (core + 1) * b_shard].reshape(b_shard, 3 * V)
        )
        in_maps.append(dict(verts=stripe, **blobs))

    res = run_bass_kernel_spmd(
        nc, in_maps, core_ids=list(range(n_cores)), trace=_trace
    )
    J = meta["J"]
    full = np.empty((B, J, 3), np.float32)
    for core in range(n_cores):
        stage = res.results[core]["out"]  # [J, 3, b_shard]
        full[core * b_shard : (core + 1) * b_shard] = stage.transpose(2, 0, 1)
    if _trace:
        kernel._last_result = res
    return full
